# revision 1
# baseline (speedup 1.0000x reference)
"""Dense bilinear spatial-transformer warp (nn_Dense3DSpatialTransformer) on 8 TRN2 cores.

Math: the reference output is, per pixel (h, w),
    out[h,w] = sum_{dy,dx in [-6,6]} img[h+dy, w+dx] * tri_h(dy) * tri_w(dx)
with tri_h(dy) = relu(1 - |H_up - (h+1+dy)|), H_up = fp32((fp32(fh + h)) + 1)
(and likewise for columns), img zero-padded outside [0,4096).  Only the 4
taps around the warped sample point have nonzero weight, so this dense
13x13-tap form is exact — including the reference's border-clipping cases,
which all read zero-padded rows/cols.  The reference's fp32 rounding of
H_up/W_up is reproduced via e = ((f + mesh) + 1) - mesh (the final subtract
is exact by Sterbenz), so tap weights match the reference bit-for-bit.

Sharding: H rows split across 8 cores (512 each).  Each core's input is a
zero-padded 524 x 4108 image band (6-row/col halo on each side), so halo
exchange happens for free at input-distribution time.

Layout: 128 partitions = column blocks of 32; free dim = (rows x 44 cols)
band so both tap shifts are static free-dim AP offsets.

Compute structure per 32-row chunk (v3 "stacked" formulation):
  ACC[j=dy-plane, :] = sum_dx b_dx (*) band_plane(j, dx)
      per dx: ONE 13-plane stacked multiply (b_dx broadcast across planes
      via a stride-0 AP dim) + ONE stacked add  -> 26 large DVE ops
  out = reduce_dy( A_stack (*) ACC )              -> 1 mul + 1 strided reduce
  tri weights (a/b) are computed on the Scalar engine (Abs then Relu
  activations), overlapping the DVE stream.
"""

import time
from contextlib import ExitStack

import numpy as np

import bass_rust
import concourse.bacc as bacc
import concourse.mybir as mybir
import concourse.tile as tile

F32 = mybir.dt.float32

H = 4096
W = 4096
NCORES = 8
SH = H // NCORES          # 512 rows per core
HALO = 6
PADW = W + 2 * HALO       # 4108
NPART = 128
CPB = W // NPART          # 32 columns per partition block
CPB_H = CPB + 2 * HALO    # 44 columns incl. halo
NTAP = 13                 # dy, dx in [-6, 6]
R_CHUNK = 32


def _band_src_ap(t, chunk, r):
    off = chunk * r * PADW
    return bass_rust.AP(
        tensor=t.ap().tensor, offset=off,
        ap=[[CPB, NPART], [PADW, r + 2 * HALO], [1, CPB_H]],
    )


def _flat_src_ap(t, chunk, r, sh_w):
    off = chunk * r * sh_w
    return bass_rust.AP(
        tensor=t.ap().tensor, offset=off,
        ap=[[CPB, NPART], [sh_w, r], [1, CPB]],
    )


def _band_stack_view(band_tile, dx, r):
    """[128, 13(dy), r, 32] view of band [128, r+12, 44]: plane j reads rows
    shifted by j, cols shifted by dx (overlapping strided AP)."""
    base = band_tile[:]
    return bass_rust.AP(
        tensor=base.tensor,
        offset=base.offset + (dx + HALO),
        ap=[list(base.ap[0]), [CPB_H, NTAP], [CPB_H, r], [1, CPB]],
    )


def _bcast_planes(ap2d, nplanes):
    """Broadcast a [128, r, c] AP across nplanes via a stride-0 plane dim."""
    return bass_rust.AP(
        tensor=ap2d.tensor, offset=ap2d.offset,
        ap=[list(ap2d.ap[0]), [0, nplanes]] + [list(d) for d in ap2d.ap[1:]],
    )


def build_nc(sh=SH, r_chunk=R_CHUNK, debug=False):
    nc = bacc.Bacc("TRN2", target_bir_lowering=False, debug=debug)
    bandr = sh + 2 * HALO
    r = r_chunk
    n_chunks = sh // r
    assert n_chunks * r == sh
    N = r * CPB

    # const APs for the ACT bias values used by the tri-weight activations
    for v in range(-7, 8):
        val = float(v)
        if (F32, val) not in nc.const_aps.aps:
            t = nc.alloc_sbuf_tensor(f"const-float32-{val}", [128, 1], F32)
            nc.gpsimd.memset(t.ap(), val)
            nc.const_aps.aps[(F32, val)] = t.ap()
    nc.all_engine_barrier()

    img = nc.dram_tensor("img", [bandr, PADW], F32, kind="ExternalInput")
    fh = nc.dram_tensor("fh", [sh, W], F32, kind="ExternalInput")
    fw = nc.dram_tensor("fw", [sh, W], F32, kind="ExternalInput")
    rowc = nc.dram_tensor("rowc", [sh, CPB], F32, kind="ExternalInput")
    colc = nc.dram_tensor("colc", [W], F32, kind="ExternalInput")
    out = nc.dram_tensor("out", [sh, W], F32, kind="ExternalOutput")

    ABS = mybir.ActivationFunctionType.Abs
    RELU = mybir.ActivationFunctionType.Relu

    with tile.TileContext(nc) as tc, ExitStack() as ctx:
        io_pool = ctx.enter_context(tc.tile_pool(name="io", bufs=2))
        w_pool = ctx.enter_context(tc.tile_pool(name="wts", bufs=2))
        s_pool = ctx.enter_context(tc.tile_pool(name="stk", bufs=1))

        for chunk in range(n_chunks):
            band = io_pool.tile([NPART, r + 2 * HALO, CPB_H], F32, tag="band")
            nc.sync.dma_start(band[:], _band_src_ap(img, chunk, r))
            fh_t = io_pool.tile([NPART, r, CPB], F32, tag="fh")
            nc.sync.dma_start(fh_t[:], _flat_src_ap(fh, chunk, r, W))
            fw_t = io_pool.tile([NPART, r, CPB], F32, tag="fw")
            nc.sync.dma_start(fw_t[:], _flat_src_ap(fw, chunk, r, W))
            rowc_t = io_pool.tile([NPART, r, CPB], F32, tag="rowc")
            nc.sync.dma_start(
                rowc_t[:],
                bass_rust.AP(tensor=rowc.ap().tensor, offset=chunk * r * CPB,
                             ap=[[0, NPART], [CPB, r], [1, CPB]]),
            )
            colc_t = io_pool.tile([NPART, r, CPB], F32, tag="colc")
            nc.sync.dma_start(
                colc_t[:],
                bass_rust.AP(tensor=colc.ap().tensor, offset=0,
                             ap=[[CPB, NPART], [0, r], [1, CPB]]),
            )

            # e = ((f + mesh) + 1) - mesh   (reference fp32 rounding preserved)
            eh = w_pool.tile([NPART, r, CPB], F32, tag="eh")
            nc.vector.tensor_add(eh[:], fh_t[:], rowc_t[:])
            nc.vector.tensor_scalar_add(eh[:], eh[:], 1.0)
            nc.vector.tensor_sub(eh[:], eh[:], rowc_t[:])

            ew = w_pool.tile([NPART, r, CPB], F32, tag="ew")
            nc.vector.tensor_add(ew[:], fw_t[:], colc_t[:])
            nc.vector.tensor_scalar_add(ew[:], ew[:], 1.0)
            nc.vector.tensor_sub(ew[:], ew[:], colc_t[:])

            acc = s_pool.tile([NPART, NTAP, r, CPB], F32, tag="acc")
            tmp = s_pool.tile([NPART, NTAP, r, CPB], F32, tag="tmp")
            b_t = w_pool.tile([NPART, r, CPB], F32, tag="b")
            out_t = w_pool.tile([NPART, r, CPB], F32, tag="out")

            for dxi in range(NTAP):
                dx = dxi - HALO
                # b_dx = relu(1 - |ew - (dx+1)|)  on the Scalar engine
                nc.scalar.activation(b_t[:], ew[:], ABS,
                                     bias=float(-(dx + 1)), scale=1.0)
                nc.scalar.activation(b_t[:], b_t[:], RELU, bias=1.0, scale=-1.0)

                bview = _bcast_planes(b_t[:], NTAP)
                if dxi == 0:
                    nc.vector.tensor_mul(acc[:], bview,
                                         _band_stack_view(band, dx, r))
                else:
                    nc.vector.tensor_mul(tmp[:], bview,
                                         _band_stack_view(band, dx, r))
                    nc.vector.tensor_add(acc[:], acc[:], tmp[:])

            # a-weights written into tmp (reused), then contract over dy
            astk = tmp
            for j in range(NTAP):
                dy = j - HALO
                nc.scalar.activation(astk[:, j], eh[:], ABS,
                                     bias=float(-(dy + 1)), scale=1.0)
                nc.scalar.activation(astk[:, j], astk[:, j], RELU,
                                     bias=1.0, scale=-1.0)
            nc.vector.tensor_mul(astk[:], astk[:], acc[:])

            m_flat = astk[:].rearrange("p j r c -> p (j r c)")
            red_view = bass_rust.AP(
                tensor=m_flat.tensor, offset=m_flat.offset,
                ap=[list(m_flat.ap[0]), [1, N], [N, NTAP]],
            )
            nc.vector.tensor_reduce(
                out_t[:].rearrange("p r c -> p (r c)"), red_view,
                mybir.AxisListType.X, mybir.AluOpType.add)

            nc.sync.dma_start(_flat_src_ap(out, chunk, r, W), out_t[:])

    nc.compile()
    return nc


def shard_inputs(input1, input2, sh=SH):
    img = np.asarray(input1, dtype=np.float32).reshape(H, W)
    flow = np.asarray(input2, dtype=np.float32).reshape(2, H, W)
    ncores = H // sh

    img_pad = np.zeros((H + 2 * HALO, PADW), dtype=np.float32)
    img_pad[HALO:H + HALO, HALO:W + HALO] = img

    colc = np.arange(W, dtype=np.float32)
    in_maps = []
    for k in range(ncores):
        h0 = k * sh
        in_maps.append({
            "img": np.ascontiguousarray(img_pad[h0:h0 + sh + 2 * HALO]),
            "fh": np.ascontiguousarray(flow[0, h0:h0 + sh]),
            "fw": np.ascontiguousarray(flow[1, h0:h0 + sh]),
            "rowc": np.broadcast_to(
                np.arange(h0, h0 + sh, dtype=np.float32)[:, None], (sh, CPB)
            ).copy(),
            "colc": colc,
        })
    return in_maps


_NC_CACHE = {}


def kernel(input1, input2):
    from concourse.bass_utils import run_bass_kernel_spmd

    key = (SH, R_CHUNK)
    if key not in _NC_CACHE:
        _NC_CACHE[key] = build_nc(sh=SH, r_chunk=R_CHUNK)
    nc = _NC_CACHE[key]
    in_maps = shard_inputs(input1, input2)

    last_err = None
    for attempt in range(3):
        try:
            res = run_bass_kernel_spmd(nc, in_maps, core_ids=list(range(NCORES)))
            break
        except Exception as e:  # transient device desync — retry
            last_err = e
            time.sleep(5.0 * (attempt + 1))
    else:
        raise last_err
    out = np.concatenate([r["out"] for r in res.results], axis=0)
    return out.reshape(1, 1, H, W).astype(np.float32)



# revision 2
# speedup vs baseline: 4.7370x; 4.7370x over previous
"""Dense bilinear spatial-transformer warp v4 — fp16 DVE + Pool offload.

Same math as the baseline 13x13-tap dense formulation, with:
  - heavy stacked tensor ops in fp16 (DVE 2x perf mode: packed 2-byte inner dim)
  - tri-weight stacks (a/b) fully materialized by the Scalar engine up front
  - POOL_DX of the 13 dx tap-columns computed on the GPSIMD engine in parallel
  - warped coords eh/ew precomputed on host (drops on-device index math + DMAs)
  - r=16 chunks, double-buffered pools end to end
"""

import time
from contextlib import ExitStack

import numpy as np

import bass_rust
import concourse.bacc as bacc
import concourse.mybir as mybir
import concourse.tile as tile

F32 = mybir.dt.float32
F16 = mybir.dt.float16

H = 4096
W = 4096
NCORES = 8
SH = H // NCORES          # 512 rows per core
HALO = 6
PADW = W + 2 * HALO       # 4108
NPART = 128
CPB = W // NPART          # 32 columns per partition block
CPB_H = CPB + 2 * HALO    # 44 columns incl. halo
NTAP = 13                 # dy, dx in [-6, 6]
R_CHUNK = 16
POOL_DX = 3               # dx tap-columns handled by the GPSIMD engine


def _band_src_ap(t, chunk, r):
    off = chunk * r * PADW
    return bass_rust.AP(
        tensor=t.ap().tensor, offset=off,
        ap=[[CPB, NPART], [PADW, r + 2 * HALO], [1, CPB_H]],
    )


def _flat_src_ap(t, chunk, r, sh_w):
    off = chunk * r * sh_w
    return bass_rust.AP(
        tensor=t.ap().tensor, offset=off,
        ap=[[CPB, NPART], [sh_w, r], [1, CPB]],
    )


def _band_stack_view(band_tile, dx, r):
    """[128, 13(dy), r, 32] view of fp16 band [128, r+12, 44]."""
    base = band_tile[:]
    return bass_rust.AP(
        tensor=base.tensor,
        offset=base.offset + (dx + HALO),
        ap=[list(base.ap[0]), [CPB_H, NTAP], [CPB_H, r], [1, CPB]],
    )


def _bcast_planes(ap2d, nplanes):
    return bass_rust.AP(
        tensor=ap2d.tensor, offset=ap2d.offset,
        ap=[list(ap2d.ap[0]), [0, nplanes]] + [list(d) for d in ap2d.ap[1:]],
    )


def build_nc(sh=SH, r_chunk=R_CHUNK, pool_dx=POOL_DX, debug=False):
    nc = bacc.Bacc("TRN2", target_bir_lowering=False, debug=debug)
    r = r_chunk
    n_chunks = sh // r
    assert n_chunks * r == sh
    N = r * CPB

    for v in range(-7, 8):
        val = float(v)
        if (F32, val) not in nc.const_aps.aps:
            t = nc.alloc_sbuf_tensor(f"const-float32-{val}", [128, 1], F32)
            nc.gpsimd.memset(t.ap(), val)
            nc.const_aps.aps[(F32, val)] = t.ap()
    nc.all_engine_barrier()

    img = nc.dram_tensor("img", [sh + 2 * HALO, PADW], F32, kind="ExternalInput")
    eh = nc.dram_tensor("eh", [sh, W], F32, kind="ExternalInput")
    ew = nc.dram_tensor("ew", [sh, W], F32, kind="ExternalInput")
    out = nc.dram_tensor("out", [sh, W], F32, kind="ExternalOutput")

    ABS = mybir.ActivationFunctionType.Abs
    RELU = mybir.ActivationFunctionType.Relu

    dve_dxs = [dxi for dxi in range(NTAP) if dxi % 5 != 2 or pool_dx == 0]
    pool_dxs = [dxi for dxi in range(NTAP) if dxi not in dve_dxs][:pool_dx]
    dve_dxs = [dxi for dxi in range(NTAP) if dxi not in pool_dxs]

    with tile.TileContext(nc) as tc, ExitStack() as ctx:
        io_pool = ctx.enter_context(tc.tile_pool(name="io", bufs=2))
        w_pool = ctx.enter_context(tc.tile_pool(name="wts", bufs=2))
        s_pool = ctx.enter_context(tc.tile_pool(name="stk", bufs=2))

        for chunk in range(n_chunks):
            band32 = io_pool.tile([NPART, r + 2 * HALO, CPB_H], F32, tag="band32")
            nc.sync.dma_start(band32[:], _band_src_ap(img, chunk, r))
            eh_t = io_pool.tile([NPART, r, CPB], F32, tag="eh")
            nc.sync.dma_start(eh_t[:], _flat_src_ap(eh, chunk, r, W))
            ew_t = io_pool.tile([NPART, r, CPB], F32, tag="ew")
            nc.sync.dma_start(ew_t[:], _flat_src_ap(ew, chunk, r, W))

            band = s_pool.tile([NPART, r + 2 * HALO, CPB_H], F16, tag="band16")
            nc.scalar.copy(band[:], band32[:])

            bstk = w_pool.tile([NPART, NTAP, r, CPB], F16, tag="bstk")
            astk = w_pool.tile([NPART, NTAP, r, CPB], F16, tag="astk")
            for dxi in range(NTAP):
                dx = dxi - HALO
                nc.scalar.activation(bstk[:, dxi], ew_t[:], ABS,
                                     bias=float(-(dx + 1)), scale=1.0)
                nc.scalar.activation(bstk[:, dxi], bstk[:, dxi], RELU,
                                     bias=1.0, scale=-1.0)
            for j in range(NTAP):
                dy = j - HALO
                nc.scalar.activation(astk[:, j], eh_t[:], ABS,
                                     bias=float(-(dy + 1)), scale=1.0)
                nc.scalar.activation(astk[:, j], astk[:, j], RELU,
                                     bias=1.0, scale=-1.0)

            acc = s_pool.tile([NPART, NTAP, r, CPB], F16, tag="acc")
            tmp = s_pool.tile([NPART, NTAP, r, CPB], F16, tag="tmp")

            if pool_dxs:
                pacc = s_pool.tile([NPART, NTAP, r, CPB], F16, tag="pacc")
                ptmp = s_pool.tile([NPART, NTAP, r, CPB], F16, tag="ptmp")
                for i, dxi in enumerate(pool_dxs):
                    dx = dxi - HALO
                    bview = _bcast_planes(bstk[:, dxi], NTAP)
                    if i == 0:
                        nc.gpsimd.tensor_mul(pacc[:], bview,
                                             _band_stack_view(band, dx, r))
                    else:
                        nc.gpsimd.tensor_mul(ptmp[:], bview,
                                             _band_stack_view(band, dx, r))
                        nc.gpsimd.tensor_add(pacc[:], pacc[:], ptmp[:])

            for i, dxi in enumerate(dve_dxs):
                dx = dxi - HALO
                bview = _bcast_planes(bstk[:, dxi], NTAP)
                if i == 0:
                    nc.vector.tensor_mul(acc[:], bview,
                                         _band_stack_view(band, dx, r))
                else:
                    nc.vector.tensor_mul(tmp[:], bview,
                                         _band_stack_view(band, dx, r))
                    nc.vector.tensor_add(acc[:], acc[:], tmp[:])

            if pool_dxs:
                nc.vector.tensor_add(acc[:], acc[:], pacc[:])

            nc.vector.tensor_mul(astk[:], astk[:], acc[:])

            o16 = w_pool.tile([NPART, r, CPB], F16, tag="o16")
            m_flat = astk[:].rearrange("p j r c -> p (j r c)")
            red_view = bass_rust.AP(
                tensor=m_flat.tensor, offset=m_flat.offset,
                ap=[list(m_flat.ap[0]), [1, N], [N, NTAP]],
            )
            with nc.allow_low_precision(reason="fp16 warp accumulate within tolerance"):
                nc.vector.tensor_reduce(
                    o16[:].rearrange("p r c -> p (r c)"), red_view,
                    mybir.AxisListType.X, mybir.AluOpType.add)

            out_t = w_pool.tile([NPART, r, CPB], F32, tag="out")
            nc.scalar.copy(out_t[:], o16[:])
            nc.sync.dma_start(_flat_src_ap(out, chunk, r, W), out_t[:])

    nc.compile()
    return nc


def shard_inputs(input1, input2, sh=SH):
    img = np.asarray(input1, dtype=np.float32).reshape(H, W)
    flow = np.asarray(input2, dtype=np.float32).reshape(2, H, W)
    ncores = H // sh
    f32 = np.float32

    img_pad = np.zeros((H + 2 * HALO, PADW), dtype=f32)
    img_pad[HALO:H + HALO, HALO:W + HALO] = img

    rowv = np.arange(H, dtype=f32)[:, None]
    colv = np.arange(W, dtype=f32)[None, :]
    # e = ((f + mesh) + 1) - mesh, every step in fp32 (matches device baseline)
    eh_full = (((flow[0] + rowv).astype(f32) + f32(1.0)).astype(f32) - rowv).astype(f32)
    ew_full = (((flow[1] + colv).astype(f32) + f32(1.0)).astype(f32) - colv).astype(f32)

    in_maps = []
    for k in range(ncores):
        h0 = k * sh
        in_maps.append({
            "img": np.ascontiguousarray(img_pad[h0:h0 + sh + 2 * HALO]),
            "eh": np.ascontiguousarray(eh_full[h0:h0 + sh]),
            "ew": np.ascontiguousarray(ew_full[h0:h0 + sh]),
        })
    return in_maps


_NC_CACHE = {}


def kernel(input1, input2):
    from concourse.bass_utils import run_bass_kernel_spmd

    key = (SH, R_CHUNK, POOL_DX)
    if key not in _NC_CACHE:
        _NC_CACHE[key] = build_nc(sh=SH, r_chunk=R_CHUNK, pool_dx=POOL_DX)
    nc = _NC_CACHE[key]
    in_maps = shard_inputs(input1, input2)

    last_err = None
    for attempt in range(3):
        try:
            res = run_bass_kernel_spmd(nc, in_maps, core_ids=list(range(NCORES)))
            break
        except Exception as e:  # transient device desync — retry
            last_err = e
            time.sleep(5.0 * (attempt + 1))
    else:
        raise last_err
    out = np.concatenate([r["out"] for r in res.results], axis=0)
    return out.reshape(1, 1, H, W).astype(np.float32)
